# revision 29
# baseline (speedup 1.0000x reference)
"""ExpertConv2d Trainium2 kernel: per-patch mixture-of-experts 3x3 conv.

Problem: x (4,64,512,512) f32 split into 256 patches of (64ch, 64x64);
each patch convolved (pad=1) with a per-patch mix of 5 expert kernels
(mix weights v), plus mixed bias.  Data-parallel over patches across 8
NeuronCores (32 patches/core, processed as 16 patch-pairs).

Device plan per core (v6):
 - The PE only does the conv.  Weight mixing runs on DVE + GpSimd as
   fused multiply-add chains (scalar_tensor_tensor), writing mixed
   per-pair weights DIRECTLY into the conv stationary layout
   w_all[ci + 64*parity, pair, tap*64+co] -- no PE mixing stream and
   no SBUF->SBUF redistribution DMAs at all.  Inputs for this are
   wfci [128, k*576 + tap*64+co] (expert weights with ci on
   partitions, duplicated to both partition halves) and vvb
   [128, pair, k] (per-patch mix scalars, partition-broadcast).
 - A couple of junk warmup matmuls un-throttle the PE HAM clock gate
   before the first conv matmuls.
 - conv: per pair, x tile [128, 4096] bf16 (A | B channel blocks).
   Per chunk (8 y-rows = 512 outputs) 9 tap-matmuls accumulate in
   PSUM; boundary taps shrink the output rectangle.  Quadrants: row
   group = patch half, col group (psum half) = patch ^ chunk parity,
   so 4 K=64/M=64 matmuls run concurrently = full PE.  Reused-weight
   matmuls get their LDWEIGHTS stripped post-hoc.
 - copyback: ACT/DVE per-partition bias add PSUM->SBUF bf16, then one
   out DMA per c4 group (0.5MB).  Host unscrambles the layout.
"""

import os
import sys

import numpy as np

sys.path.insert(0, "/opt/trn_rl_repo")

import concourse.bass as bass  # noqa: E402
import concourse.tile as tile  # noqa: E402
from concourse import mybir  # noqa: E402

import bass_rust as _bass_rust  # noqa: E402

# ---------------------------------------------------------------------------
# Workaround: this walrus build rejects >1 sync-wait on one instruction.
# TileContext._drain_and_barrier attaches one wait per live sem lane to a
# single SP Drain.  Replace it: one SP wait_ge per lane, then a clean drain.
# ---------------------------------------------------------------------------


def _split_drain_and_barrier(self, tick_clock, wait_clock):
    nc = self.nc
    gc = tick_clock.global_clock
    assert self.sems is not None
    allocated = self.sems.allocated()
    for proc, sem in sorted(allocated.items()):
        t = gc[proc] if proc < len(gc) else 0
        if t > 0:
            nc.sync.wait_ge(sem, _bass_rust.tick_to_sem(t, proc))
    nc.sync.drain()
    nc.all_engine_barrier()
    popped = nc._tile_sem_poison_stack.pop()
    assert popped is self._sem_poison
    nc.clear_and_free_semaphores(list(allocated.values()))
    nc.all_engine_barrier()


tile.TileContext._drain_and_barrier = _split_drain_and_barrier

_MAX_WAITS = 1


def _split_excess_waits(nc):
    """Walrus (CoreV2/V3 setupSyncWait) accepts at most 1 sem-wait on a
    Matmult.  Tile can attach more.  Move the excess onto NoOps inserted
    immediately before the instruction on the same engine (same queue order,
    so semantics are unchanged)."""
    n_split = 0
    for fn in nc.m.functions:
        for bb in fn.blocks:
            insts = list(bb.instructions)
            out = []
            changed = False
            for inst in insts:
                si = inst.sync_info
                waits = list(si.on_wait) if si is not None and si.on_wait else []
                if len(waits) > _MAX_WAITS:
                    keep = waits[-_MAX_WAITS:]
                    excess = waits[:-_MAX_WAITS]
                    for i in range(0, len(excess), _MAX_WAITS):
                        grp = excess[i:i + _MAX_WAITS]
                        nop = mybir.InstNoOp(
                            name=f"{inst.name}_wsplit{i}", ins=[], outs=[])
                        nop.engine = inst.engine
                        nop.sync_info = mybir.SyncInfo(on_wait=grp, on_update=[])
                        out.append(nop)
                    inst.sync_info = mybir.SyncInfo(
                        on_wait=keep,
                        on_update=list(si.on_update) if si.on_update else [])
                    changed = True
                    n_split += 1
                out.append(inst)
            if changed:
                bb.instructions = out
    return n_split


def _strip_reuse_ldweights(nc, reuse_names):
    """Remove the InstLdweights paired with matmuls whose stationary operand
    is already loaded in their PE quadrant (same weights loaded earlier).
    Merges the ldweights' sync info into the matmul so no ordering edges are
    lost."""
    n = 0
    for fn in nc.m.functions:
        for bb in fn.blocks:
            insts = list(bb.instructions)
            out = []
            changed = False
            k = 0
            while k < len(insts):
                inst = insts[k]
                nxt = insts[k + 1] if k + 1 < len(insts) else None
                if (isinstance(inst, mybir.InstLdweights)
                        and nxt is not None
                        and isinstance(nxt, mybir.InstMatmult)
                        and nxt.name in reuse_names):
                    lsi = inst.sync_info
                    if lsi is not None and (lsi.on_wait or lsi.on_update):
                        msi = nxt.sync_info
                        mw = list(msi.on_wait) if msi and msi.on_wait else []
                        mu = list(msi.on_update) if msi and msi.on_update else []
                        nxt.sync_info = mybir.SyncInfo(
                            on_wait=list(lsi.on_wait or []) + mw,
                            on_update=mu + list(lsi.on_update or []))
                    changed = True
                    n += 1
                    k += 1
                    continue
                out.append(inst)
                k += 1
            if changed:
                bb.instructions = out
    return n


# ---------------------------------------------------------------------------
# Constants (hardcoded problem shape)
# ---------------------------------------------------------------------------
B, C_IN, C_OUT, K, KS, P_SZ, HW = 4, 64, 64, 5, 3, 64, 512
GRID = HW // P_SZ                  # 8x8 patch grid
N_CORES = 8
N_PATCH = B * GRID * GRID          # 256
PPC = N_PATCH // N_CORES           # 32 patches per core
NPAIR = PPC // 2                   # 16 pairs per core
NCHUNK = 8                         # 512-wide output chunks per patch
TFREE = KS * KS * C_OUT            # 576 = (tap, co) per-patch weight cols
BF16 = mybir.dt.bfloat16
F32 = mybir.dt.float32

N_WARM = 4                         # junk matmuls to un-throttle the PE HAM
DVE_SLICE = 384                    # DVE mixes cols [0:384), gpsimd [384:576)
MIX_AHEAD = 4                      # pairs premixed before the conv loop

_NC_CACHE = {}
_REUSE_MM_NAMES = set()


def _tap_geometry(c, ky, kx):
    """Output sub-rectangle of chunk c covered by tap (ky, kx) and the
    matching input offset.  Returns None if empty (never happens here)."""
    y0 = max(8 * c, 1 - ky)
    y1 = min(8 * c + 8, P_SZ + 1 - ky)
    x0 = max(0, 1 - kx)
    x1 = min(P_SZ, P_SZ + 1 - kx)
    if y0 >= y1 or x0 >= x1:
        return None
    in_off = (y0 + ky - 1) * P_SZ + (x0 + kx - 1)
    out_off = (y0 - 8 * c) * P_SZ + x0
    return in_off, out_off, y1 - y0, x1 - x0


def build_nc(npair=NPAIR, split_waits=True, warm=True):
    nc = bass.Bass("TRN2")
    xin = nc.dram_tensor("xin", [npair, 128, P_SZ * P_SZ], BF16, kind="ExternalInput")
    wfci = nc.dram_tensor("wfci", [128, K * TFREE], BF16, kind="ExternalInput")
    vvb = nc.dram_tensor("vvb", [128, npair, K], F32, kind="ExternalInput")
    vvlo = nc.dram_tensor("vvlo", [K, 2 * npair], BF16, kind="ExternalInput")
    vvhi = nc.dram_tensor("vvhi", [K, 2 * npair], BF16, kind="ExternalInput")
    bbank = nc.dram_tensor("bbank", [K, C_OUT], BF16, kind="ExternalInput")
    out = nc.dram_tensor("out", [npair, 128, P_SZ * P_SZ], BF16, kind="ExternalOutput")

    npatch = 2 * npair
    with tile.TileContext(nc) as tc:
        with (
            tc.tile_pool(name="persist", bufs=1) as persist,
            tc.tile_pool(name="xpool", bufs=6) as xpool,
            tc.tile_pool(name="opool", bufs=5) as opool,
            tc.tile_pool(name="psum", bufs=8, space="PSUM") as pp,
        ):
            # ---- expert bank + mix scalars: sync/scalar queues, head ----
            wfci_sb = persist.tile([128, K * TFREE], BF16)
            HALF = K * TFREE // 2
            nc.sync.dma_start(out=wfci_sb[:, 0:HALF], in_=wfci[:, 0:HALF])
            nc.scalar.dma_start(out=wfci_sb[:, HALF:], in_=wfci[:, HALF:])

            # ---- small constants on gpsimd ----
            vvb_sb = persist.tile([128, npair, K], F32)
            nc.gpsimd.dma_start(out=vvb_sb, in_=vvb[:, :, :])
            vvlo_sb = persist.tile([K, npatch], BF16)
            nc.gpsimd.dma_start(out=vvlo_sb, in_=vvlo[:, :])
            vvhi_sb = persist.tile([K, npatch], BF16)
            nc.gpsimd.dma_start(out=vvhi_sb, in_=vvhi[:, :])
            bbank_sb = persist.tile([K, C_OUT], BF16)
            nc.gpsimd.dma_start(out=bbank_sb, in_=bbank[:, :])

            # ---- PE warmup: junk matmuls so HAM un-throttles early ----
            warm_x = persist.tile([128, 512], BF16)
            nc.vector.memset(warm_x, 0.0)
            if warm:
                warm_ps = pp.tile([128, 512], F32, tag="pc", name="warm")
                for i in range(N_WARM):
                    nc.tensor.matmul(warm_ps, lhsT=warm_x[:, 0:128], rhs=warm_x,
                                     start=True, stop=True)

            # ---- bias mixing:  bias_sb[0:64, p] = bias(patch p),
            #      bias_sb[64:128, p] = bias(pair-swapped p) ----
            psum_b = pp.tile([128, npatch], F32, tag="pc", name="psum_b")
            nc.tensor.matmul(psum_b[0:64, :], lhsT=bbank_sb, rhs=vvlo_sb,
                             start=True, stop=True, skip_group_check=True)
            nc.tensor.matmul(psum_b[64:128, :], lhsT=bbank_sb, rhs=vvhi_sb,
                             start=True, stop=True, skip_group_check=True)
            bias_sb = persist.tile([128, npatch], F32)
            nc.vector.tensor_copy(bias_sb, psum_b)

            # ---- weight mixing on DVE + GpSimd, straight into the conv
            #      stationary layout.  w_all[ci+64*par, j, t*64+co] =
            #      sum_k vvb[., j, k] * wfci[., k*576 + t*64+co] ----
            w_all = persist.tile([128, npair, TFREE], BF16)
            mul = mybir.AluOpType.mult
            add = mybir.AluOpType.add

            def mix_pair(j):
                # DVE only: walrus rejects TensorScalarPtr on Pool/gpsimd
                eng = nc.vector
                acc = w_all[:, j, :]
                eng.tensor_scalar_mul(
                    acc, wfci_sb[:, 0:TFREE], vvb_sb[:, j, 0:1])
                for k in range(1, K):
                    eng.scalar_tensor_tensor(
                        acc, wfci_sb[:, k * TFREE:(k + 1) * TFREE],
                        vvb_sb[:, j, k:k + 1], acc, op0=mul, op1=add)

            for j in range(min(MIX_AHEAD, npair)):
                mix_pair(j)

            # ---- main loop over pairs ----
            taps = [(1, 1)] + [(ky, kx) for ky in range(KS) for kx in range(KS)
                               if (ky, kx) != (1, 1)]
            for j in range(npair):
                x_t = xpool.tile([128, P_SZ * P_SZ], BF16, tag="x")
                nc.sync.dma_start(out=x_t, in_=xin[j, :, :])
                o_t = opool.tile([128, P_SZ * P_SZ], BF16, tag="o")
                if j + MIX_AHEAD < npair:
                    mix_pair(j + MIX_AHEAD)

                for c4 in range(NCHUNK // 4):
                    chunks = tuple(4 * c4 + i for i in range(4))
                    psums = {}
                    for c in chunks:
                        psums[c] = pp.tile([128, 512], F32, tag="pc", name="pc")
                    for ti, (ky, kx) in enumerate(taps):
                        first = ti == 0
                        last = ti == len(taps) - 1
                        # Order so the 4 in-flight matmuls cover 4 distinct
                        # PSUM banks and all 4 PE quadrants; second wave reuses
                        # each quadrant's already-loaded weights.
                        order = [(chunks[0], 0, False), (chunks[1], 0, False),
                                 (chunks[2], 1, False), (chunks[3], 1, False),
                                 (chunks[2], 0, True), (chunks[3], 0, True),
                                 (chunks[0], 1, True), (chunks[1], 1, True)]
                        for c, P, reuse in order:
                            h = P ^ (c & 1)
                            in_off, out_off, cy, cx = _tap_geometry(c, ky, kx)
                            y_in0 = in_off // P_SZ
                            x_in0 = in_off % P_SZ
                            rhs = x_t[64 * P:64 * P + 64, :].rearrange(
                                "p (y x) -> p y x", x=P_SZ)[
                                :, y_in0:y_in0 + cy, x_in0:x_in0 + cx]
                            y_o0 = out_off // P_SZ
                            x_o0 = out_off % P_SZ
                            outap = psums[c][64 * h:64 * h + 64, :].rearrange(
                                "p (y x) -> p y x", x=P_SZ)[
                                :, y_o0:y_o0 + cy, x_o0:x_o0 + cx]
                            t = ky * KS + kx
                            lhsT = w_all[64 * P:64 * P + 64, j,
                                         t * C_OUT:(t + 1) * C_OUT]
                            mi = nc.tensor.matmul(outap, lhsT=lhsT, rhs=rhs,
                                                  start=first, stop=last,
                                                  skip_group_check=True)
                            if reuse:
                                _REUSE_MM_NAMES.add(mi.ins.name)
                    for c in chunks:
                        col = 2 * j + (c & 1)
                        dst = o_t[:, c * 512:(c + 1) * 512]
                        if c & 1:
                            nc.vector.tensor_scalar_add(
                                dst, psums[c], bias_sb[:, col:col + 1])
                        else:
                            nc.scalar.activation(
                                dst, psums[c],
                                mybir.ActivationFunctionType.Identity,
                                bias=bias_sb[:, col:col + 1], scale=1.0)
                    # pipelined output: one 0.5MB DMA per c4 group.  Scalar
                    # queue (sync must stay free for xin prefetch), except
                    # the last pairs where sync is done with xin and the
                    # scalar queue backlog would stretch the tail.
                    half_f = P_SZ * P_SZ // 2
                    eng = nc.sync if j >= npair - 2 else nc.scalar
                    eng.dma_start(
                        out=out[j, :, c4 * half_f:(c4 + 1) * half_f],
                        in_=o_t[:, c4 * half_f:(c4 + 1) * half_f])
    ns = _strip_reuse_ldweights(nc, _REUSE_MM_NAMES)
    if split_waits:
        n = _split_excess_waits(nc)
        if n:
            print(f"[kernel] split {n} waits; stripped {ns} ldweights")
    return nc


# ---------------------------------------------------------------------------
# Host marshalling
# ---------------------------------------------------------------------------


def _marshal_inputs(x, v, weight, bias):
    import ml_dtypes

    bf16 = ml_dtypes.bfloat16
    # x: (B, C, 512, 512) -> per patch (b, gy, gx) blocks of [64, 64, 64]
    xp = x.reshape(B, C_IN, GRID, P_SZ, GRID, P_SZ)
    xp = xp.transpose(0, 2, 4, 1, 3, 5)          # b gy gx ci y x
    xp = np.ascontiguousarray(xp).reshape(N_PATCH, C_IN, P_SZ * P_SZ)
    # per core: [NPAIR, 128(=2 patches x ci), 4096]
    xc = xp.reshape(N_CORES, NPAIR, 2 * C_IN, P_SZ * P_SZ)

    # vv: (b, k, gy, gx) -> [patch, k]
    vv = v.transpose(0, 2, 3, 1).reshape(N_PATCH, K)
    vvc = vv.reshape(N_CORES, PPC, K)
    vv_lo = vvc.transpose(0, 2, 1).astype(bf16)              # [core, K, 32]
    swap = vvc.reshape(N_CORES, NPAIR, 2, K)[:, :, ::-1, :]
    vv_hi = swap.reshape(N_CORES, PPC, K).transpose(0, 2, 1).astype(bf16)

    # vvb[p, j, k] = vv[pair j, parity p>=64, k], partition-broadcast
    vvp = vvc.reshape(N_CORES, NPAIR, 2, K)                  # core j par k
    vvb = np.repeat(vvp.transpose(0, 2, 1, 3), C_IN, axis=1)  # core 128 j k
    vvb = np.ascontiguousarray(vvb).astype(np.float32)

    # wfci[p, k*576 + t*64 + co] = weight[k, co, p%64, ky, kx]
    wt = weight.transpose(2, 0, 3, 4, 1)          # ci k ky kx co
    wt = np.ascontiguousarray(wt).reshape(C_IN, K * TFREE)
    wfci = np.tile(wt, (2, 1)).astype(bf16)       # [128, K*576]

    bb = bias.astype(bf16)                                   # [k, co]

    in_maps = []
    for m in range(N_CORES):
        in_maps.append({
            "xin": np.ascontiguousarray(xc[m]).astype(bf16),
            "wfci": wfci,
            "vvb": vvb[m],
            "vvlo": np.ascontiguousarray(vv_lo[m]),
            "vvhi": np.ascontiguousarray(vv_hi[m]),
            "bbank": bb,
        })
    return in_maps


def _unmarshal_output(dev_outs):
    """dev_outs: list of 8 arrays [NPAIR, 128, 4096] f32 -> (B, C_OUT, 512, 512)."""
    out = np.empty((B, C_OUT, HW, HW), np.float32)
    patches = np.empty((N_PATCH, C_OUT, P_SZ, P_SZ), np.float32)
    for m in range(N_CORES):
        a = dev_outs[m].astype(np.float32).reshape(NPAIR, 2, C_OUT, 4, 2, 8, P_SZ)
        # axes: j, h, co, c2, cp, yy, x ; patch_local = h ^ cp
        p0 = a[:, :, :, :, 0]                      # cp=0: patch = h
        p1 = a[:, ::-1, :, :, 1]                   # cp=1: patch = 1-h
        b = np.stack([p0, p1], axis=4)             # j, patch, co, c2, cp, yy, x
        b = b.reshape(NPAIR, 2, C_OUT, NCHUNK * 8, P_SZ)  # y = (c2, cp, yy)
        patches[m * PPC:(m + 1) * PPC] = b.reshape(PPC, C_OUT, P_SZ, P_SZ)
    pt = patches.reshape(B, GRID, GRID, C_OUT, P_SZ, P_SZ)
    out = pt.transpose(0, 3, 1, 4, 2, 5).reshape(B, C_OUT, HW, HW)
    return np.ascontiguousarray(out)


def kernel(x, v, weight, bias, trace=False):
    from concourse.bass_utils import run_bass_kernel_spmd

    x = np.asarray(x, dtype=np.float32)
    v = np.asarray(v, dtype=np.float32)
    weight = np.asarray(weight, dtype=np.float32)
    bias = np.asarray(bias, dtype=np.float32)

    if "nc" not in _NC_CACHE:
        _NC_CACHE["nc"] = build_nc()
    nc = _NC_CACHE["nc"]

    in_maps = _marshal_inputs(x, v, weight, bias)
    res = run_bass_kernel_spmd(nc, in_maps, core_ids=list(range(N_CORES)),
                               trace=trace)
    dev_outs = [res.results[m]["out"] for m in range(N_CORES)]
    full = _unmarshal_output(dev_outs)
    kernel.last_result = res
    return full


# revision 32
# speedup vs baseline: 1.0148x; 1.0148x over previous
"""ExpertConv2d Trainium2 kernel: per-patch mixture-of-experts 3x3 conv.

Problem: x (4,64,512,512) f32 split into 256 patches of (64ch, 64x64);
each patch convolved (pad=1) with a per-patch mix of 5 expert kernels
(mix weights v), plus mixed bias.  Data-parallel over patches across 8
NeuronCores (32 patches/core, processed as 16 patch-pairs).

Device plan per core (v6):
 - The PE only does the conv.  Weight mixing runs on DVE + GpSimd as
   fused multiply-add chains (scalar_tensor_tensor), writing mixed
   per-pair weights DIRECTLY into the conv stationary layout
   w_all[ci + 64*parity, pair, tap*64+co] -- no PE mixing stream and
   no SBUF->SBUF redistribution DMAs at all.  Inputs for this are
   wfci [128, k*576 + tap*64+co] (expert weights with ci on
   partitions, duplicated to both partition halves) and vvb
   [128, pair, k] (per-patch mix scalars, partition-broadcast).
 - A couple of junk warmup matmuls un-throttle the PE HAM clock gate
   before the first conv matmuls.
 - conv: per pair, x tile [128, 4096] bf16 (A | B channel blocks).
   Per chunk (8 y-rows = 512 outputs) 9 tap-matmuls accumulate in
   PSUM; boundary taps shrink the output rectangle.  Quadrants: row
   group = patch half, col group (psum half) = patch ^ chunk parity,
   so 4 K=64/M=64 matmuls run concurrently = full PE.  Reused-weight
   matmuls get their LDWEIGHTS stripped post-hoc.
 - copyback: ACT/DVE per-partition bias add PSUM->SBUF bf16, then one
   out DMA per c4 group (0.5MB).  Host unscrambles the layout.
"""

import os
import sys

import numpy as np

sys.path.insert(0, "/opt/trn_rl_repo")

import concourse.bass as bass  # noqa: E402
import concourse.tile as tile  # noqa: E402
from concourse import mybir  # noqa: E402

import bass_rust as _bass_rust  # noqa: E402

# ---------------------------------------------------------------------------
# Workaround: this walrus build rejects >1 sync-wait on one instruction.
# TileContext._drain_and_barrier attaches one wait per live sem lane to a
# single SP Drain.  Replace it: one SP wait_ge per lane, then a clean drain.
# ---------------------------------------------------------------------------


def _split_drain_and_barrier(self, tick_clock, wait_clock):
    nc = self.nc
    gc = tick_clock.global_clock
    assert self.sems is not None
    allocated = self.sems.allocated()
    for proc, sem in sorted(allocated.items()):
        t = gc[proc] if proc < len(gc) else 0
        if t > 0:
            nc.sync.wait_ge(sem, _bass_rust.tick_to_sem(t, proc))
    nc.sync.drain()
    nc.all_engine_barrier()
    popped = nc._tile_sem_poison_stack.pop()
    assert popped is self._sem_poison
    nc.clear_and_free_semaphores(list(allocated.values()))
    nc.all_engine_barrier()


tile.TileContext._drain_and_barrier = _split_drain_and_barrier

_MAX_WAITS = 1


def _split_excess_waits(nc):
    """Walrus (CoreV2/V3 setupSyncWait) accepts at most 1 sem-wait on a
    Matmult.  Tile can attach more.  Move the excess onto NoOps inserted
    immediately before the instruction on the same engine (same queue order,
    so semantics are unchanged)."""
    n_split = 0
    for fn in nc.m.functions:
        for bb in fn.blocks:
            insts = list(bb.instructions)
            out = []
            changed = False
            for inst in insts:
                si = inst.sync_info
                waits = list(si.on_wait) if si is not None and si.on_wait else []
                if len(waits) > _MAX_WAITS:
                    keep = waits[-_MAX_WAITS:]
                    excess = waits[:-_MAX_WAITS]
                    for i in range(0, len(excess), _MAX_WAITS):
                        grp = excess[i:i + _MAX_WAITS]
                        nop = mybir.InstNoOp(
                            name=f"{inst.name}_wsplit{i}", ins=[], outs=[])
                        nop.engine = inst.engine
                        nop.sync_info = mybir.SyncInfo(on_wait=grp, on_update=[])
                        out.append(nop)
                    inst.sync_info = mybir.SyncInfo(
                        on_wait=keep,
                        on_update=list(si.on_update) if si.on_update else [])
                    changed = True
                    n_split += 1
                out.append(inst)
            if changed:
                bb.instructions = out
    return n_split


def _strip_reuse_ldweights(nc, reuse_names):
    """Remove the InstLdweights paired with matmuls whose stationary operand
    is already loaded in their PE quadrant (same weights loaded earlier).
    Merges the ldweights' sync info into the matmul so no ordering edges are
    lost."""
    n = 0
    for fn in nc.m.functions:
        for bb in fn.blocks:
            insts = list(bb.instructions)
            out = []
            changed = False
            k = 0
            while k < len(insts):
                inst = insts[k]
                nxt = insts[k + 1] if k + 1 < len(insts) else None
                if (isinstance(inst, mybir.InstLdweights)
                        and nxt is not None
                        and isinstance(nxt, mybir.InstMatmult)
                        and nxt.name in reuse_names):
                    lsi = inst.sync_info
                    if lsi is not None and (lsi.on_wait or lsi.on_update):
                        msi = nxt.sync_info
                        mw = list(msi.on_wait) if msi and msi.on_wait else []
                        mu = list(msi.on_update) if msi and msi.on_update else []
                        nxt.sync_info = mybir.SyncInfo(
                            on_wait=list(lsi.on_wait or []) + mw,
                            on_update=mu + list(lsi.on_update or []))
                    changed = True
                    n += 1
                    k += 1
                    continue
                out.append(inst)
                k += 1
            if changed:
                bb.instructions = out
    return n


# ---------------------------------------------------------------------------
# Constants (hardcoded problem shape)
# ---------------------------------------------------------------------------
B, C_IN, C_OUT, K, KS, P_SZ, HW = 4, 64, 64, 5, 3, 64, 512
GRID = HW // P_SZ                  # 8x8 patch grid
N_CORES = 8
N_PATCH = B * GRID * GRID          # 256
PPC = N_PATCH // N_CORES           # 32 patches per core
NPAIR = PPC // 2                   # 16 pairs per core
NCHUNK = 8                         # 512-wide output chunks per patch
TFREE = KS * KS * C_OUT            # 576 = (tap, co) per-patch weight cols
BF16 = mybir.dt.bfloat16
F32 = mybir.dt.float32

N_WARM = 4                         # junk matmuls to un-throttle the PE HAM
DVE_SLICE = 384                    # DVE mixes cols [0:384), gpsimd [384:576)
MIX_AHEAD = 2                      # pairs premixed before the conv loop

_NC_CACHE = {}
_REUSE_MM_NAMES = set()


def _tap_geometry(c, ky, kx):
    """Output sub-rectangle of chunk c covered by tap (ky, kx) and the
    matching input offset.  Returns None if empty (never happens here)."""
    y0 = max(8 * c, 1 - ky)
    y1 = min(8 * c + 8, P_SZ + 1 - ky)
    x0 = max(0, 1 - kx)
    x1 = min(P_SZ, P_SZ + 1 - kx)
    if y0 >= y1 or x0 >= x1:
        return None
    in_off = (y0 + ky - 1) * P_SZ + (x0 + kx - 1)
    out_off = (y0 - 8 * c) * P_SZ + x0
    return in_off, out_off, y1 - y0, x1 - x0


def build_nc(npair=NPAIR, split_waits=True, warm=True):
    nc = bass.Bass("TRN2")
    xin = nc.dram_tensor("xin", [npair, 128, P_SZ * P_SZ], BF16, kind="ExternalInput")
    wfci = nc.dram_tensor("wfci", [128, K * TFREE], BF16, kind="ExternalInput")
    vvb = nc.dram_tensor("vvb", [128, npair, K], F32, kind="ExternalInput")
    vvlo = nc.dram_tensor("vvlo", [K, 2 * npair], BF16, kind="ExternalInput")
    vvhi = nc.dram_tensor("vvhi", [K, 2 * npair], BF16, kind="ExternalInput")
    bbank = nc.dram_tensor("bbank", [K, C_OUT], BF16, kind="ExternalInput")
    out = nc.dram_tensor("out", [npair, 128, P_SZ * P_SZ], BF16, kind="ExternalOutput")

    npatch = 2 * npair
    with tile.TileContext(nc) as tc:
        with (
            tc.tile_pool(name="persist", bufs=1) as persist,
            tc.tile_pool(name="xpool", bufs=6) as xpool,
            tc.tile_pool(name="opool", bufs=5) as opool,
            tc.tile_pool(name="psum", bufs=8, space="PSUM") as pp,
        ):
            # ---- expert bank + mix scalars: sync/scalar queues, head ----
            wfci_sb = persist.tile([128, K * TFREE], BF16)
            HALF = K * TFREE // 2
            nc.sync.dma_start(out=wfci_sb[:, 0:HALF], in_=wfci[:, 0:HALF])
            nc.scalar.dma_start(out=wfci_sb[:, HALF:], in_=wfci[:, HALF:])

            # ---- small constants on gpsimd ----
            vvb_sb = persist.tile([128, npair, K], F32)
            nc.gpsimd.dma_start(out=vvb_sb, in_=vvb[:, :, :])
            vvlo_sb = persist.tile([K, npatch], BF16)
            nc.gpsimd.dma_start(out=vvlo_sb, in_=vvlo[:, :])
            vvhi_sb = persist.tile([K, npatch], BF16)
            nc.gpsimd.dma_start(out=vvhi_sb, in_=vvhi[:, :])
            bbank_sb = persist.tile([K, C_OUT], BF16)
            nc.gpsimd.dma_start(out=bbank_sb, in_=bbank[:, :])

            # ---- PE warmup: junk matmuls so HAM un-throttles early ----
            warm_x = persist.tile([128, 512], BF16)
            nc.vector.memset(warm_x, 0.0)
            if warm:
                warm_ps = pp.tile([128, 512], F32, tag="pc", name="warm")
                for i in range(N_WARM):
                    nc.tensor.matmul(warm_ps, lhsT=warm_x[:, 0:128], rhs=warm_x,
                                     start=True, stop=True)

            # ---- bias mixing:  bias_sb[0:64, p] = bias(patch p),
            #      bias_sb[64:128, p] = bias(pair-swapped p) ----
            psum_b = pp.tile([128, npatch], F32, tag="pc", name="psum_b")
            nc.tensor.matmul(psum_b[0:64, :], lhsT=bbank_sb, rhs=vvlo_sb,
                             start=True, stop=True, skip_group_check=True)
            nc.tensor.matmul(psum_b[64:128, :], lhsT=bbank_sb, rhs=vvhi_sb,
                             start=True, stop=True, skip_group_check=True)
            bias_sb = persist.tile([128, npatch], F32)
            nc.scalar.copy(out=bias_sb, in_=psum_b)

            # ---- weight mixing on DVE + GpSimd, straight into the conv
            #      stationary layout.  w_all[ci+64*par, j, t*64+co] =
            #      sum_k vvb[., j, k] * wfci[., k*576 + t*64+co] ----
            w_all = persist.tile([128, npair, TFREE], BF16)
            mul = mybir.AluOpType.mult
            add = mybir.AluOpType.add

            def mix_pair(j):
                # DVE only: walrus rejects TensorScalarPtr on Pool/gpsimd
                eng = nc.vector
                acc = w_all[:, j, :]
                eng.tensor_scalar_mul(
                    acc, wfci_sb[:, 0:TFREE], vvb_sb[:, j, 0:1])
                for k in range(1, K):
                    eng.scalar_tensor_tensor(
                        acc, wfci_sb[:, k * TFREE:(k + 1) * TFREE],
                        vvb_sb[:, j, k:k + 1], acc, op0=mul, op1=add)

            for j in range(min(MIX_AHEAD, npair)):
                mix_pair(j)

            # ---- main loop over pairs ----
            taps = [(1, 1)] + [(ky, kx) for ky in range(KS) for kx in range(KS)
                               if (ky, kx) != (1, 1)]
            for j in range(npair):
                x_t = xpool.tile([128, P_SZ * P_SZ], BF16, tag="x")
                nc.sync.dma_start(out=x_t, in_=xin[j, :, :])
                o_t = opool.tile([128, P_SZ * P_SZ], BF16, tag="o")
                if j + MIX_AHEAD < npair:
                    mix_pair(j + MIX_AHEAD)

                for c4 in range(NCHUNK // 4):
                    chunks = tuple(4 * c4 + i for i in range(4))
                    psums = {}
                    for c in chunks:
                        psums[c] = pp.tile([128, 512], F32, tag="pc", name="pc")
                    for ti, (ky, kx) in enumerate(taps):
                        first = ti == 0
                        last = ti == len(taps) - 1
                        # Order so the 4 in-flight matmuls cover 4 distinct
                        # PSUM banks and all 4 PE quadrants; second wave reuses
                        # each quadrant's already-loaded weights.
                        order = [(chunks[0], 0, False), (chunks[1], 0, False),
                                 (chunks[2], 1, False), (chunks[3], 1, False),
                                 (chunks[2], 0, True), (chunks[3], 0, True),
                                 (chunks[0], 1, True), (chunks[1], 1, True)]
                        for c, P, reuse in order:
                            h = P ^ (c & 1)
                            in_off, out_off, cy, cx = _tap_geometry(c, ky, kx)
                            y_in0 = in_off // P_SZ
                            x_in0 = in_off % P_SZ
                            rhs = x_t[64 * P:64 * P + 64, :].rearrange(
                                "p (y x) -> p y x", x=P_SZ)[
                                :, y_in0:y_in0 + cy, x_in0:x_in0 + cx]
                            y_o0 = out_off // P_SZ
                            x_o0 = out_off % P_SZ
                            outap = psums[c][64 * h:64 * h + 64, :].rearrange(
                                "p (y x) -> p y x", x=P_SZ)[
                                :, y_o0:y_o0 + cy, x_o0:x_o0 + cx]
                            t = ky * KS + kx
                            lhsT = w_all[64 * P:64 * P + 64, j,
                                         t * C_OUT:(t + 1) * C_OUT]
                            mi = nc.tensor.matmul(outap, lhsT=lhsT, rhs=rhs,
                                                  start=first, stop=last,
                                                  skip_group_check=True)
                            if reuse:
                                _REUSE_MM_NAMES.add(mi.ins.name)
                    # DVE carries the mixing chains, so it only gets one
                    # copyback chunk per c4 group; ACT takes the rest.
                    for c in chunks:
                        col = 2 * j + (c & 1)
                        dst = o_t[:, c * 512:(c + 1) * 512]
                        if c % 4 == 3:
                            nc.vector.tensor_scalar_add(
                                dst, psums[c], bias_sb[:, col:col + 1])
                        else:
                            nc.scalar.activation(
                                dst, psums[c],
                                mybir.ActivationFunctionType.Identity,
                                bias=bias_sb[:, col:col + 1], scale=1.0)
                    # pipelined output: one 0.5MB DMA per c4 group.  Scalar
                    # queue (sync must stay free for xin prefetch), except
                    # the last pairs where sync is done with xin and the
                    # scalar queue backlog would stretch the tail.
                    half_f = P_SZ * P_SZ // 2
                    eng = nc.sync if j >= npair - 2 else nc.scalar
                    eng.dma_start(
                        out=out[j, :, c4 * half_f:(c4 + 1) * half_f],
                        in_=o_t[:, c4 * half_f:(c4 + 1) * half_f])
    ns = _strip_reuse_ldweights(nc, _REUSE_MM_NAMES)
    if split_waits:
        n = _split_excess_waits(nc)
        if n:
            print(f"[kernel] split {n} waits; stripped {ns} ldweights")
    return nc


# ---------------------------------------------------------------------------
# Host marshalling
# ---------------------------------------------------------------------------


def _marshal_inputs(x, v, weight, bias):
    import ml_dtypes

    bf16 = ml_dtypes.bfloat16
    # x: (B, C, 512, 512) -> per patch (b, gy, gx) blocks of [64, 64, 64]
    xp = x.reshape(B, C_IN, GRID, P_SZ, GRID, P_SZ)
    xp = xp.transpose(0, 2, 4, 1, 3, 5)          # b gy gx ci y x
    xp = np.ascontiguousarray(xp).reshape(N_PATCH, C_IN, P_SZ * P_SZ)
    # per core: [NPAIR, 128(=2 patches x ci), 4096]
    xc = xp.reshape(N_CORES, NPAIR, 2 * C_IN, P_SZ * P_SZ)

    # vv: (b, k, gy, gx) -> [patch, k]
    vv = v.transpose(0, 2, 3, 1).reshape(N_PATCH, K)
    vvc = vv.reshape(N_CORES, PPC, K)
    vv_lo = vvc.transpose(0, 2, 1).astype(bf16)              # [core, K, 32]
    swap = vvc.reshape(N_CORES, NPAIR, 2, K)[:, :, ::-1, :]
    vv_hi = swap.reshape(N_CORES, PPC, K).transpose(0, 2, 1).astype(bf16)

    # vvb[p, j, k] = vv[pair j, parity p>=64, k], partition-broadcast
    vvp = vvc.reshape(N_CORES, NPAIR, 2, K)                  # core j par k
    vvb = np.repeat(vvp.transpose(0, 2, 1, 3), C_IN, axis=1)  # core 128 j k
    vvb = np.ascontiguousarray(vvb).astype(np.float32)

    # wfci[p, k*576 + t*64 + co] = weight[k, co, p%64, ky, kx]
    wt = weight.transpose(2, 0, 3, 4, 1)          # ci k ky kx co
    wt = np.ascontiguousarray(wt).reshape(C_IN, K * TFREE)
    wfci = np.tile(wt, (2, 1)).astype(bf16)       # [128, K*576]

    bb = bias.astype(bf16)                                   # [k, co]

    in_maps = []
    for m in range(N_CORES):
        in_maps.append({
            "xin": np.ascontiguousarray(xc[m]).astype(bf16),
            "wfci": wfci,
            "vvb": vvb[m],
            "vvlo": np.ascontiguousarray(vv_lo[m]),
            "vvhi": np.ascontiguousarray(vv_hi[m]),
            "bbank": bb,
        })
    return in_maps


def _unmarshal_output(dev_outs):
    """dev_outs: list of 8 arrays [NPAIR, 128, 4096] f32 -> (B, C_OUT, 512, 512)."""
    out = np.empty((B, C_OUT, HW, HW), np.float32)
    patches = np.empty((N_PATCH, C_OUT, P_SZ, P_SZ), np.float32)
    for m in range(N_CORES):
        a = dev_outs[m].astype(np.float32).reshape(NPAIR, 2, C_OUT, 4, 2, 8, P_SZ)
        # axes: j, h, co, c2, cp, yy, x ; patch_local = h ^ cp
        p0 = a[:, :, :, :, 0]                      # cp=0: patch = h
        p1 = a[:, ::-1, :, :, 1]                   # cp=1: patch = 1-h
        b = np.stack([p0, p1], axis=4)             # j, patch, co, c2, cp, yy, x
        b = b.reshape(NPAIR, 2, C_OUT, NCHUNK * 8, P_SZ)  # y = (c2, cp, yy)
        patches[m * PPC:(m + 1) * PPC] = b.reshape(PPC, C_OUT, P_SZ, P_SZ)
    pt = patches.reshape(B, GRID, GRID, C_OUT, P_SZ, P_SZ)
    out = pt.transpose(0, 3, 1, 4, 2, 5).reshape(B, C_OUT, HW, HW)
    return np.ascontiguousarray(out)


def kernel(x, v, weight, bias, trace=False):
    from concourse.bass_utils import run_bass_kernel_spmd

    x = np.asarray(x, dtype=np.float32)
    v = np.asarray(v, dtype=np.float32)
    weight = np.asarray(weight, dtype=np.float32)
    bias = np.asarray(bias, dtype=np.float32)

    if "nc" not in _NC_CACHE:
        _NC_CACHE["nc"] = build_nc()
    nc = _NC_CACHE["nc"]

    in_maps = _marshal_inputs(x, v, weight, bias)
    res = run_bass_kernel_spmd(nc, in_maps, core_ids=list(range(N_CORES)),
                               trace=trace)
    dev_outs = [res.results[m]["out"] for m in range(N_CORES)]
    full = _unmarshal_output(dev_outs)
    kernel.last_result = res
    return full


# revision 33
# speedup vs baseline: 1.0231x; 1.0081x over previous
"""ExpertConv2d Trainium2 kernel: per-patch mixture-of-experts 3x3 conv.

Problem: x (4,64,512,512) f32 split into 256 patches of (64ch, 64x64);
each patch convolved (pad=1) with a per-patch mix of 5 expert kernels
(mix weights v), plus mixed bias.  Data-parallel over patches across 8
NeuronCores (32 patches/core, processed as 16 patch-pairs).

Device plan per core (v6):
 - The PE only does the conv.  Weight mixing runs on DVE + GpSimd as
   fused multiply-add chains (scalar_tensor_tensor), writing mixed
   per-pair weights DIRECTLY into the conv stationary layout
   w_all[ci + 64*parity, pair, tap*64+co] -- no PE mixing stream and
   no SBUF->SBUF redistribution DMAs at all.  Inputs for this are
   wfci [128, k*576 + tap*64+co] (expert weights with ci on
   partitions, duplicated to both partition halves) and vvb
   [128, pair, k] (per-patch mix scalars, partition-broadcast).
 - A couple of junk warmup matmuls un-throttle the PE HAM clock gate
   before the first conv matmuls.
 - conv: per pair, x tile [128, 4096] bf16 (A | B channel blocks).
   Per chunk (8 y-rows = 512 outputs) 9 tap-matmuls accumulate in
   PSUM; boundary taps shrink the output rectangle.  Quadrants: row
   group = patch half, col group (psum half) = patch ^ chunk parity,
   so 4 K=64/M=64 matmuls run concurrently = full PE.  Reused-weight
   matmuls get their LDWEIGHTS stripped post-hoc.
 - copyback: ACT/DVE per-partition bias add PSUM->SBUF bf16, then one
   out DMA per c4 group (0.5MB).  Host unscrambles the layout.
"""

import os
import sys

import numpy as np

sys.path.insert(0, "/opt/trn_rl_repo")

import concourse.bass as bass  # noqa: E402
import concourse.tile as tile  # noqa: E402
from concourse import mybir  # noqa: E402

import bass_rust as _bass_rust  # noqa: E402

# ---------------------------------------------------------------------------
# Workaround: this walrus build rejects >1 sync-wait on one instruction.
# TileContext._drain_and_barrier attaches one wait per live sem lane to a
# single SP Drain.  Replace it: one SP wait_ge per lane, then a clean drain.
# ---------------------------------------------------------------------------


def _split_drain_and_barrier(self, tick_clock, wait_clock):
    nc = self.nc
    gc = tick_clock.global_clock
    assert self.sems is not None
    allocated = self.sems.allocated()
    for proc, sem in sorted(allocated.items()):
        t = gc[proc] if proc < len(gc) else 0
        if t > 0:
            nc.sync.wait_ge(sem, _bass_rust.tick_to_sem(t, proc))
    nc.sync.drain()
    nc.all_engine_barrier()
    popped = nc._tile_sem_poison_stack.pop()
    assert popped is self._sem_poison
    nc.clear_and_free_semaphores(list(allocated.values()))
    nc.all_engine_barrier()


tile.TileContext._drain_and_barrier = _split_drain_and_barrier

_MAX_WAITS = 1


def _split_excess_waits(nc):
    """Walrus (CoreV2/V3 setupSyncWait) accepts at most 1 sem-wait on a
    Matmult.  Tile can attach more.  Move the excess onto NoOps inserted
    immediately before the instruction on the same engine (same queue order,
    so semantics are unchanged)."""
    n_split = 0
    for fn in nc.m.functions:
        for bb in fn.blocks:
            insts = list(bb.instructions)
            out = []
            changed = False
            for inst in insts:
                si = inst.sync_info
                waits = list(si.on_wait) if si is not None and si.on_wait else []
                if len(waits) > _MAX_WAITS:
                    keep = waits[-_MAX_WAITS:]
                    excess = waits[:-_MAX_WAITS]
                    for i in range(0, len(excess), _MAX_WAITS):
                        grp = excess[i:i + _MAX_WAITS]
                        nop = mybir.InstNoOp(
                            name=f"{inst.name}_wsplit{i}", ins=[], outs=[])
                        nop.engine = inst.engine
                        nop.sync_info = mybir.SyncInfo(on_wait=grp, on_update=[])
                        out.append(nop)
                    inst.sync_info = mybir.SyncInfo(
                        on_wait=keep,
                        on_update=list(si.on_update) if si.on_update else [])
                    changed = True
                    n_split += 1
                out.append(inst)
            if changed:
                bb.instructions = out
    return n_split


def _strip_reuse_ldweights(nc, reuse_names):
    """Remove the InstLdweights paired with matmuls whose stationary operand
    is already loaded in their PE quadrant (same weights loaded earlier).
    Merges the ldweights' sync info into the matmul so no ordering edges are
    lost."""
    n = 0
    for fn in nc.m.functions:
        for bb in fn.blocks:
            insts = list(bb.instructions)
            out = []
            changed = False
            k = 0
            while k < len(insts):
                inst = insts[k]
                nxt = insts[k + 1] if k + 1 < len(insts) else None
                if (isinstance(inst, mybir.InstLdweights)
                        and nxt is not None
                        and isinstance(nxt, mybir.InstMatmult)
                        and nxt.name in reuse_names):
                    lsi = inst.sync_info
                    if lsi is not None and (lsi.on_wait or lsi.on_update):
                        msi = nxt.sync_info
                        mw = list(msi.on_wait) if msi and msi.on_wait else []
                        mu = list(msi.on_update) if msi and msi.on_update else []
                        nxt.sync_info = mybir.SyncInfo(
                            on_wait=list(lsi.on_wait or []) + mw,
                            on_update=mu + list(lsi.on_update or []))
                    changed = True
                    n += 1
                    k += 1
                    continue
                out.append(inst)
                k += 1
            if changed:
                bb.instructions = out
    return n


# ---------------------------------------------------------------------------
# Constants (hardcoded problem shape)
# ---------------------------------------------------------------------------
B, C_IN, C_OUT, K, KS, P_SZ, HW = 4, 64, 64, 5, 3, 64, 512
GRID = HW // P_SZ                  # 8x8 patch grid
N_CORES = 8
N_PATCH = B * GRID * GRID          # 256
PPC = N_PATCH // N_CORES           # 32 patches per core
NPAIR = PPC // 2                   # 16 pairs per core
NCHUNK = 8                         # 512-wide output chunks per patch
TFREE = KS * KS * C_OUT            # 576 = (tap, co) per-patch weight cols
BF16 = mybir.dt.bfloat16
F32 = mybir.dt.float32

N_WARM = 4                         # junk matmuls to un-throttle the PE HAM
DVE_SLICE = 384                    # DVE mixes cols [0:384), gpsimd [384:576)
MIX_AHEAD = 2                      # pairs premixed before the conv loop

_NC_CACHE = {}
_REUSE_MM_NAMES = set()


def _tap_geometry(c, ky, kx):
    """Output sub-rectangle of chunk c covered by tap (ky, kx) and the
    matching input offset.  Returns None if empty (never happens here)."""
    y0 = max(8 * c, 1 - ky)
    y1 = min(8 * c + 8, P_SZ + 1 - ky)
    x0 = max(0, 1 - kx)
    x1 = min(P_SZ, P_SZ + 1 - kx)
    if y0 >= y1 or x0 >= x1:
        return None
    in_off = (y0 + ky - 1) * P_SZ + (x0 + kx - 1)
    out_off = (y0 - 8 * c) * P_SZ + x0
    return in_off, out_off, y1 - y0, x1 - x0


def build_nc(npair=NPAIR, split_waits=True, warm=True):
    nc = bass.Bass("TRN2")
    xin = nc.dram_tensor("xin", [npair, 128, P_SZ * P_SZ], BF16, kind="ExternalInput")
    wfci = nc.dram_tensor("wfci", [128, K * TFREE], BF16, kind="ExternalInput")
    vvb = nc.dram_tensor("vvb", [128, npair, K], F32, kind="ExternalInput")
    vvlo = nc.dram_tensor("vvlo", [K, 2 * npair], BF16, kind="ExternalInput")
    vvhi = nc.dram_tensor("vvhi", [K, 2 * npair], BF16, kind="ExternalInput")
    bbank = nc.dram_tensor("bbank", [K, C_OUT], BF16, kind="ExternalInput")
    out = nc.dram_tensor("out", [npair, 128, P_SZ * P_SZ], BF16, kind="ExternalOutput")

    npatch = 2 * npair
    with tile.TileContext(nc) as tc:
        with (
            tc.tile_pool(name="persist", bufs=1) as persist,
            tc.tile_pool(name="xpool", bufs=6) as xpool,
            tc.tile_pool(name="opool", bufs=5) as opool,
            tc.tile_pool(name="psum", bufs=8, space="PSUM") as pp,
        ):
            # ---- expert bank + mix scalars: sync/scalar queues, head ----
            wfci_sb = persist.tile([128, K * TFREE], BF16)
            HALF = K * TFREE // 2
            nc.sync.dma_start(out=wfci_sb[:, 0:HALF], in_=wfci[:, 0:HALF])
            nc.scalar.dma_start(out=wfci_sb[:, HALF:], in_=wfci[:, HALF:])

            # ---- small constants on gpsimd ----
            vvb_sb = persist.tile([128, npair, K], F32)
            nc.gpsimd.dma_start(out=vvb_sb, in_=vvb[:, :, :])
            vvlo_sb = persist.tile([K, npatch], BF16)
            nc.gpsimd.dma_start(out=vvlo_sb, in_=vvlo[:, :])
            vvhi_sb = persist.tile([K, npatch], BF16)
            nc.gpsimd.dma_start(out=vvhi_sb, in_=vvhi[:, :])
            bbank_sb = persist.tile([K, C_OUT], BF16)
            nc.gpsimd.dma_start(out=bbank_sb, in_=bbank[:, :])

            # ---- PE warmup: junk matmuls so HAM un-throttles early ----
            warm_x = persist.tile([128, 512], BF16)
            nc.vector.memset(warm_x, 0.0)
            if warm:
                warm_ps = pp.tile([128, 512], F32, tag="pc", name="warm")
                for i in range(N_WARM):
                    nc.tensor.matmul(warm_ps, lhsT=warm_x[:, 0:128], rhs=warm_x,
                                     start=True, stop=True)

            # ---- bias mixing:  bias_sb[0:64, p] = bias(patch p),
            #      bias_sb[64:128, p] = bias(pair-swapped p) ----
            psum_b = pp.tile([128, npatch], F32, tag="pc", name="psum_b")
            nc.tensor.matmul(psum_b[0:64, :], lhsT=bbank_sb, rhs=vvlo_sb,
                             start=True, stop=True, skip_group_check=True)
            nc.tensor.matmul(psum_b[64:128, :], lhsT=bbank_sb, rhs=vvhi_sb,
                             start=True, stop=True, skip_group_check=True)
            bias_sb = persist.tile([128, npatch], F32)
            nc.scalar.copy(out=bias_sb, in_=psum_b)

            # ---- weight mixing on DVE + GpSimd, straight into the conv
            #      stationary layout.  w_all[ci+64*par, j, t*64+co] =
            #      sum_k vvb[., j, k] * wfci[., k*576 + t*64+co] ----
            w_all = persist.tile([128, npair, TFREE], BF16)
            mul = mybir.AluOpType.mult
            add = mybir.AluOpType.add

            def mix_pair(j):
                # DVE only: walrus rejects TensorScalarPtr on Pool/gpsimd.
                # Deprioritized so the scheduler slots PSUM copybacks (which
                # free banks the PE is waiting on) ahead of mixing work.
                eng = nc.vector
                acc = w_all[:, j, :]
                with tc.high_priority(offset=-1000000):
                    eng.tensor_scalar_mul(
                        acc, wfci_sb[:, 0:TFREE], vvb_sb[:, j, 0:1])
                    for k in range(1, K):
                        eng.scalar_tensor_tensor(
                            acc, wfci_sb[:, k * TFREE:(k + 1) * TFREE],
                            vvb_sb[:, j, k:k + 1], acc, op0=mul, op1=add)

            for j in range(min(MIX_AHEAD, npair)):
                mix_pair(j)

            # ---- main loop over pairs ----
            taps = [(1, 1)] + [(ky, kx) for ky in range(KS) for kx in range(KS)
                               if (ky, kx) != (1, 1)]
            for j in range(npair):
                x_t = xpool.tile([128, P_SZ * P_SZ], BF16, tag="x")
                nc.sync.dma_start(out=x_t, in_=xin[j, :, :])
                o_t = opool.tile([128, P_SZ * P_SZ], BF16, tag="o")
                if j + MIX_AHEAD < npair:
                    mix_pair(j + MIX_AHEAD)

                for c4 in range(NCHUNK // 4):
                    chunks = tuple(4 * c4 + i for i in range(4))
                    psums = {}
                    for c in chunks:
                        psums[c] = pp.tile([128, 512], F32, tag="pc", name="pc")
                    for ti, (ky, kx) in enumerate(taps):
                        first = ti == 0
                        last = ti == len(taps) - 1
                        # Order so the 4 in-flight matmuls cover 4 distinct
                        # PSUM banks and all 4 PE quadrants; second wave reuses
                        # each quadrant's already-loaded weights.
                        order = [(chunks[0], 0, False), (chunks[1], 0, False),
                                 (chunks[2], 1, False), (chunks[3], 1, False),
                                 (chunks[2], 0, True), (chunks[3], 0, True),
                                 (chunks[0], 1, True), (chunks[1], 1, True)]
                        for c, P, reuse in order:
                            h = P ^ (c & 1)
                            in_off, out_off, cy, cx = _tap_geometry(c, ky, kx)
                            y_in0 = in_off // P_SZ
                            x_in0 = in_off % P_SZ
                            rhs = x_t[64 * P:64 * P + 64, :].rearrange(
                                "p (y x) -> p y x", x=P_SZ)[
                                :, y_in0:y_in0 + cy, x_in0:x_in0 + cx]
                            y_o0 = out_off // P_SZ
                            x_o0 = out_off % P_SZ
                            outap = psums[c][64 * h:64 * h + 64, :].rearrange(
                                "p (y x) -> p y x", x=P_SZ)[
                                :, y_o0:y_o0 + cy, x_o0:x_o0 + cx]
                            t = ky * KS + kx
                            lhsT = w_all[64 * P:64 * P + 64, j,
                                         t * C_OUT:(t + 1) * C_OUT]
                            mi = nc.tensor.matmul(outap, lhsT=lhsT, rhs=rhs,
                                                  start=first, stop=last,
                                                  skip_group_check=True)
                            if reuse:
                                _REUSE_MM_NAMES.add(mi.ins.name)
                    # DVE carries the mixing chains, so it only gets one
                    # copyback chunk per c4 group; ACT takes the rest.
                    for c in chunks:
                        col = 2 * j + (c & 1)
                        dst = o_t[:, c * 512:(c + 1) * 512]
                        if c % 4 == 3:
                            nc.vector.tensor_scalar_add(
                                dst, psums[c], bias_sb[:, col:col + 1])
                        else:
                            nc.scalar.activation(
                                dst, psums[c],
                                mybir.ActivationFunctionType.Identity,
                                bias=bias_sb[:, col:col + 1], scale=1.0)
                    # pipelined output: one 0.5MB DMA per c4 group.  Scalar
                    # queue (sync must stay free for xin prefetch), except
                    # the last pairs where sync is done with xin and the
                    # scalar queue backlog would stretch the tail.
                    half_f = P_SZ * P_SZ // 2
                    eng = nc.sync if j >= npair - 2 else nc.scalar
                    eng.dma_start(
                        out=out[j, :, c4 * half_f:(c4 + 1) * half_f],
                        in_=o_t[:, c4 * half_f:(c4 + 1) * half_f])
    ns = _strip_reuse_ldweights(nc, _REUSE_MM_NAMES)
    if split_waits:
        n = _split_excess_waits(nc)
        if n:
            print(f"[kernel] split {n} waits; stripped {ns} ldweights")
    return nc


# ---------------------------------------------------------------------------
# Host marshalling
# ---------------------------------------------------------------------------


def _marshal_inputs(x, v, weight, bias):
    import ml_dtypes

    bf16 = ml_dtypes.bfloat16
    # x: (B, C, 512, 512) -> per patch (b, gy, gx) blocks of [64, 64, 64]
    xp = x.reshape(B, C_IN, GRID, P_SZ, GRID, P_SZ)
    xp = xp.transpose(0, 2, 4, 1, 3, 5)          # b gy gx ci y x
    xp = np.ascontiguousarray(xp).reshape(N_PATCH, C_IN, P_SZ * P_SZ)
    # per core: [NPAIR, 128(=2 patches x ci), 4096]
    xc = xp.reshape(N_CORES, NPAIR, 2 * C_IN, P_SZ * P_SZ)

    # vv: (b, k, gy, gx) -> [patch, k]
    vv = v.transpose(0, 2, 3, 1).reshape(N_PATCH, K)
    vvc = vv.reshape(N_CORES, PPC, K)
    vv_lo = vvc.transpose(0, 2, 1).astype(bf16)              # [core, K, 32]
    swap = vvc.reshape(N_CORES, NPAIR, 2, K)[:, :, ::-1, :]
    vv_hi = swap.reshape(N_CORES, PPC, K).transpose(0, 2, 1).astype(bf16)

    # vvb[p, j, k] = vv[pair j, parity p>=64, k], partition-broadcast
    vvp = vvc.reshape(N_CORES, NPAIR, 2, K)                  # core j par k
    vvb = np.repeat(vvp.transpose(0, 2, 1, 3), C_IN, axis=1)  # core 128 j k
    vvb = np.ascontiguousarray(vvb).astype(np.float32)

    # wfci[p, k*576 + t*64 + co] = weight[k, co, p%64, ky, kx]
    wt = weight.transpose(2, 0, 3, 4, 1)          # ci k ky kx co
    wt = np.ascontiguousarray(wt).reshape(C_IN, K * TFREE)
    wfci = np.tile(wt, (2, 1)).astype(bf16)       # [128, K*576]

    bb = bias.astype(bf16)                                   # [k, co]

    in_maps = []
    for m in range(N_CORES):
        in_maps.append({
            "xin": np.ascontiguousarray(xc[m]).astype(bf16),
            "wfci": wfci,
            "vvb": vvb[m],
            "vvlo": np.ascontiguousarray(vv_lo[m]),
            "vvhi": np.ascontiguousarray(vv_hi[m]),
            "bbank": bb,
        })
    return in_maps


def _unmarshal_output(dev_outs):
    """dev_outs: list of 8 arrays [NPAIR, 128, 4096] f32 -> (B, C_OUT, 512, 512)."""
    out = np.empty((B, C_OUT, HW, HW), np.float32)
    patches = np.empty((N_PATCH, C_OUT, P_SZ, P_SZ), np.float32)
    for m in range(N_CORES):
        a = dev_outs[m].astype(np.float32).reshape(NPAIR, 2, C_OUT, 4, 2, 8, P_SZ)
        # axes: j, h, co, c2, cp, yy, x ; patch_local = h ^ cp
        p0 = a[:, :, :, :, 0]                      # cp=0: patch = h
        p1 = a[:, ::-1, :, :, 1]                   # cp=1: patch = 1-h
        b = np.stack([p0, p1], axis=4)             # j, patch, co, c2, cp, yy, x
        b = b.reshape(NPAIR, 2, C_OUT, NCHUNK * 8, P_SZ)  # y = (c2, cp, yy)
        patches[m * PPC:(m + 1) * PPC] = b.reshape(PPC, C_OUT, P_SZ, P_SZ)
    pt = patches.reshape(B, GRID, GRID, C_OUT, P_SZ, P_SZ)
    out = pt.transpose(0, 3, 1, 4, 2, 5).reshape(B, C_OUT, HW, HW)
    return np.ascontiguousarray(out)


def kernel(x, v, weight, bias, trace=False):
    from concourse.bass_utils import run_bass_kernel_spmd

    x = np.asarray(x, dtype=np.float32)
    v = np.asarray(v, dtype=np.float32)
    weight = np.asarray(weight, dtype=np.float32)
    bias = np.asarray(bias, dtype=np.float32)

    if "nc" not in _NC_CACHE:
        _NC_CACHE["nc"] = build_nc()
    nc = _NC_CACHE["nc"]

    in_maps = _marshal_inputs(x, v, weight, bias)
    res = run_bass_kernel_spmd(nc, in_maps, core_ids=list(range(N_CORES)),
                               trace=trace)
    dev_outs = [res.results[m]["out"] for m in range(N_CORES)]
    full = _unmarshal_output(dev_outs)
    kernel.last_result = res
    return full
